# revision 21
# baseline (speedup 1.0000x reference)
"""Multi-head attention layer on 8 Trainium2 NeuronCores — v2.

Sharding (zero-communication): core c -> (batch c//2, head-group c%2); each
core owns 8 of 16 heads for one batch element.  Host sums the two partial
row-parallel out-projections per batch and adds bo + bv @ Wo.T.

v2 over the ~500us baseline:
  * LDWEIGHTS dedup: post-compile pass deletes redundant InstLdweights
    (identical stationary already resident in the same PE tile region,
    wait-free, write-once whitelist).  The compiler re-emits an LDW per
    matmul (~120ns serialized each; ~180us/core total on the baseline).
  * ACT is the floor engine (~342us of exps).  Emission is software-
    pipelined so it never starves: section A (l in [0,1024)) runs c-major
    4-st sessions; each session's AV matmuls are deferred and emitted
    under the NEXT session's exps, then partial AV+sums evict to SBUF
    accumulators, freeing PSUM so K/Q/V projection chunks interleave in
    dedicated slots instead of a 139us serial prologue.
  * Sections B1/B2 (l in [1024,1536),[1536,2048)): 512-wide pairs with both
    heads' scores in one PSUM tile (one exp per st); out-projection of
    earlier sections drains at pair boundaries, shrinking the 52us tail.

dtypes: matmul operands fp16; PSUM/softmax fp32; attT/P fp16; the section-A
AV accumulators are bf16 (range for unnormalized sums, SBUF budget).

Measured: 432.2us HW exec, max rel err 2.1e-3 (gate 2e-2); same-session
baseline measured 570.8us.  Iteration history: 443.8 (sessions+dedup) ->
439.5 (B-section AV stream trails exps by 3 steps; boundary out-proj drains
move after the next pair's first two exps) -> 438.2 (late-A session loads
rebalanced to <=1 K/Q pass each) -> 432.2 (4 out-staging bufs, 8 DRAM ring
slots for the normalize round-trips).  ACT busy ~283us is the floor; the
rest is ~30us fixed NEFF-start barrier, ~16us prologue ramp, ~9us
session-0/1 K+Q deadline stall, ~15us B-boundary residue, ~35us tail.
"""

import os
import numpy as np

B, L, S = 4, 2048, 2048
D, NH, E = 1024, 16, 64
N_CORES = 8
HG = 2
LH = NH // HG          # 8 local heads
DH = LH * E            # 512
SCALE = 1.0 / np.sqrt(E)

AW = 1024              # section A width  [0, 1024)
BW = 512               # sections B1/B2   [1024,1536) [1536,2048)

_compiled = {}
last_exec_time_ns = None
last_results = None


def _ldw_dedup(nc, mybir):
    """Delete redundant InstLdweights from the compiled module.

    The PE array keeps stationary weights until the next LDWEIGHTS touching
    the same tile region.  An LDW is deletable when (a) an earlier LDW in
    the same block already loaded the identical AP into the identical
    region with no intervening overlapping load, (b) it carries no
    semaphore waits/updates, (c) the stationary is a write-once resident
    tensor (never rewritten between the two loads).
    """
    white = ("kT", "v1", "wqr", "wkr", "attT")
    deleted = 0
    for fn in nc.m.functions:
        for bb in fn.blocks:
            insts = bb.instructions
            state = []          # (r0, r1, c0, c1, key)
            todel = []
            for idx, inst in enumerate(insts):
                if isinstance(inst, mybir.InstLdweights):
                    try:
                        w0 = inst.ins[0]
                        memref = str(w0.memref)
                        tp = inst.tile_position or (0, 0)
                        ts = inst.tile_size
                        if ts is None:
                            r0, c0, r1, c1 = 0, 0, 128, 128
                        else:
                            r0, c0 = tp
                            r1, c1 = r0 + ts[0], c0 + ts[1]
                        key = (memref, w0.offset, str(w0.ap), str(w0.dtype),
                               (r0, r1, c0, c1))
                        si = inst.sync_info
                        free = si is None or (
                            len(si.on_wait) == 0 and len(si.on_update) == 0)
                        ok = any(memref.startswith(p) for p in white)
                        if free and ok and any(e[4] == key for e in state):
                            todel.append(idx)
                            continue
                        state = [e for e in state
                                 if not (e[0] < r1 and r0 < e[1]
                                         and e[2] < c1 and c0 < e[3])]
                        state.append((r0, r1, c0, c1, key))
                    except Exception:
                        state = []
                elif isinstance(inst, mybir.InstMatmult):
                    if inst.is_transpose:
                        state = []
            for idx in reversed(todel):
                del insts[idx]
            deleted += len(todel)
    return deleted


def _build():
    import concourse.bass as bass
    import concourse.mybir as mybir
    import concourse.tile as tile
    from concourse import bacc

    f32 = mybir.dt.float32
    f16 = mybir.dt.float16
    bf16 = mybir.dt.bfloat16

    nc = bacc.Bacc("TRN2", target_bir_lowering=False, debug=False,
                   num_devices=N_CORES)

    xqT = nc.dram_tensor("xqT", [D, L], f16, kind="ExternalInput").ap()
    xkT = nc.dram_tensor("xkT", [D, S], f16, kind="ExternalInput").ap()
    xvT = nc.dram_tensor("xvT", [D, S], f16, kind="ExternalInput").ap()
    wqT = nc.dram_tensor("wqT", [D, DH], f16, kind="ExternalInput").ap()
    wkT = nc.dram_tensor("wkT", [D, DH], f16, kind="ExternalInput").ap()
    wvT = nc.dram_tensor("wvT", [D, DH], f16, kind="ExternalInput").ap()
    woT = nc.dram_tensor("woT", [DH, D], f16, kind="ExternalInput").ap()
    bq_d = nc.dram_tensor("bq", [DH], f32, kind="ExternalInput").ap()
    bk_d = nc.dram_tensor("bk", [DH], f32, kind="ExternalInput").ap()
    out_d = nc.dram_tensor("out", [L, D], f32, kind="ExternalOutput").ap()

    Exp = mybir.ActivationFunctionType.Exp
    ADD = mybir.AluOpType.add

    with tile.TileContext(nc) as tc:
        with (
            tc.tile_pool(name="res", bufs=1) as res,
            tc.tile_pool(name="xsk", bufs=32) as xsk,
            tc.tile_pool(name="xsv", bufs=8) as xsv,
            tc.tile_pool(name="pp", bufs=12) as pp,
            tc.tile_pool(name="os", bufs=4) as osp,
            tc.tile_pool(name="sm", bufs=1) as sm,
            tc.tile_pool(name="sm2", bufs=2) as sm2,
            tc.tile_pool(name="dr", bufs=8, space="DRAM") as dr,
            tc.tile_pool(name="psA", bufs=2, space="PSUM") as psA,
            tc.tile_pool(name="psV", bufs=2, space="PSUM") as psV,
            tc.tile_pool(name="psP", bufs=2, space="PSUM") as psP,
        ):
            # ---- resident weights / constants ----
            bq_sb = res.tile([128, DH // 128], f32, tag="bq")
            bk_sb = res.tile([128, DH // 128], f32, tag="bk")
            nc.sync.dma_start(bq_sb[:], bq_d.rearrange("(c p) -> p c", p=128))
            nc.sync.dma_start(bk_sb[:], bk_d.rearrange("(c p) -> p c", p=128))
            wo_sb = res.tile([128, DH // 128, D], f16, tag="wo")
            wv_sb = res.tile([128, D // 128, DH], f16, tag="wv")
            wq_r = res.tile([128, D // 128, DH], f16, tag="wqr", name="wqr")
            nc.sync.dma_start(wq_r[:], wqT.rearrange("(c p) n -> p c n", p=128))
            wk_r = res.tile([128, D // 128, DH], f16, tag="wkr", name="wkr")
            nc.sync.dma_start(wk_r[:], wkT.rearrange("(c p) n -> p c n", p=128))
            ones_f = res.tile([128, 128], f32, tag="onesf")
            nc.vector.memset(ones_f[:], 1.0)

            qT_sb = res.tile([128, DH // 128, L], f16, tag="qT", name="qT")
            kT_sb = res.tile([128, DH // 128, S], f16, tag="kT", name="kT")
            v1_sb = res.tile([128, S // 128, LH, E + 1], f16, tag="v1",
                             name="v1")
            nc.vector.tensor_copy(
                v1_sb[:, :, :, E:E + 1],
                ones_f[:, 0:S // 128 * LH].rearrange(
                    "p (s h o) -> p s h o", h=LH, o=1))

            attT = {
                "A": res.tile([128, DH // 128, AW], f16, tag="attTA",
                              name="attTA"),
                "B1": res.tile([128, DH // 128, BW], f16, tag="attTB1",
                               name="attTB1"),
                "B2": res.tile([128, DH // 128, BW], f16, tag="attTB2",
                               name="attTB2"),
            }
            # section-A AV accumulators (values rows 0:E, sums row E)
            avsb = [[res.tile([E + 1, AW], bf16, tag=f"avsb{c}{h}",
                              name=f"avsb{c}{h}")
                     for h in range(2)] for c in range(4)]

            # ---- x staging ----
            xcache = {}

            def load_x(src, nm, d, bl):
                k = (nm, d, bl)
                if k not in xcache:
                    pool = xsv if nm == "v" else xsk
                    t = pool.tile([128, 512], f16, tag="xs", name="xt")
                    nc.sync.dma_start(
                        t[:], src[d * 128:(d + 1) * 128,
                                  bl * 512:(bl + 1) * 512])
                    xcache[k] = t
                return xcache[k]

            def drop_x(nm, bls):
                for d in range(8):
                    for bl in bls:
                        xcache.pop((nm, d, bl), None)

            # ---- fused K/Q projection: one dhc chunk over a bl pair ----
            def kq_pass(src, nm, w_r, b_sb, dst, dhc, bls, drop=False):
                prj = [psP.tile([128, 512], f32, tag="prj", name="prj")
                       for _ in bls]
                for d in range(8):
                    for i, bl in enumerate(bls):
                        xt = load_x(src, nm, d, bl)
                        nc.tensor.matmul(
                            prj[i][:], w_r[:, d, dhc * 128:(dhc + 1) * 128],
                            xt[:], start=(d == 0), stop=(d == 7))
                for i, bl in enumerate(bls):
                    nc.vector.tensor_scalar_add(
                        out=dst[:, dhc, bl * 512:(bl + 1) * 512],
                        in0=prj[i][:], scalar1=b_sb[:, dhc:dhc + 1])
                if drop:
                    drop_x(nm, bls)

            # ---- V projection: one st tile (128 tokens, all local heads) --
            def v_group(st):
                bl = st // 4
                vp = psP.tile([128, DH], f32, tag="prj", name="vp")
                for d in range(8):
                    xt = load_x(xvT, "v", d, bl)
                    nc.tensor.matmul(
                        vp[:], xt[:, (st % 4) * 128:(st % 4 + 1) * 128],
                        wv_sb[:, d, :], start=(d == 0), stop=(d == 7))
                nc.vector.tensor_copy(
                    out=v1_sb[:, st, :, 0:E],
                    in_=vp.rearrange("p (h e) -> p h e", h=LH))
                if st % 4 == 3:
                    drop_x("v", (bl,))

            # ---- section A scores + exp (two separate head tiles) ----
            def sc_exp_A(c, st):
                sc0 = psA.tile([128, AW], f32, tag="A", name="sc0")
                sc1 = psA.tile([128, AW], f32, tag="A", name="sc1")
                for nh in range(AW // 512):
                    lo = nh * 512
                    nc.tensor.matmul(
                        sc0[:, lo:lo + 512],
                        kT_sb[0:64, c, st * 128:(st + 1) * 128],
                        qT_sb[0:64, c, lo:lo + 512], start=True, stop=True)
                    nc.tensor.matmul(
                        sc1[:, lo:lo + 512],
                        kT_sb[64:128, c, st * 128:(st + 1) * 128],
                        qT_sb[64:128, c, lo:lo + 512], start=True, stop=True)
                P0 = pp.tile([128, AW], f16, tag="P", name="P0")
                nc.scalar.activation(P0[:], sc0[:], Exp, scale=SCALE)
                P1 = pp.tile([128, AW], f16, tag="P", name="P1")
                nc.scalar.activation(P1[:], sc1[:], Exp, scale=SCALE)
                return P0, P1

            def avs_half(c, stb, plist, half):
                # AV accumulation for l-columns [half*512,(half+1)*512) over
                # the session's 4 st tiles, then evict-accumulate to SBUF.
                o = half * 512
                av0 = psV.tile([E + 1, 512], f32, tag="av", name="av0")
                av1 = psV.tile([E + 1, 512], f32, tag="av", name="av1")
                n = len(plist)
                for i in range(n):
                    st, P0, P1 = plist[i]
                    fs, ls = (i == 0), (i == n - 1)
                    nc.tensor.matmul(
                        av0[:], v1_sb[:, st, 2 * c, :],
                        P0[:, o:o + 512], start=fs, stop=ls)
                    nc.tensor.matmul(
                        av1[:], v1_sb[:, st, 2 * c + 1, :],
                        P1[:, o:o + 512], start=fs, stop=ls)
                for h, av in ((0, av0), (1, av1)):
                    if stb == 0:
                        nc.vector.tensor_copy(
                            avsb[c][h][:, o:o + 512], av[:])
                    else:
                        nc.vector.tensor_tensor(
                            out=avsb[c][h][:, o:o + 512],
                            in0=avsb[c][h][:, o:o + 512], in1=av[:],
                            op=ADD)

            # ---- normalize: attT[po:po+64, c, :] = av[0:E] / av[E] ----
            def normalize(sec, c, h, av_tile, width):
                po = h * 64
                sums0 = sm.tile([1, AW], f32, tag="sums", name="sums0")
                nc.vector.tensor_copy(sums0[:, 0:width], av_tile[E:E + 1, :])
                rec = sm.tile([1, AW], f32, tag="rec", name="rec")
                scr = sm.tile([1, AW], f32, tag="scr", name="scr")
                nc.vector.reciprocal_approx_accurate(
                    rec[:, 0:width], sums0[:, 0:width], scr[:, 0:width])
                rec_d = dr.tile([AW], f32, tag="recd", name="rec_d")
                nc.sync.dma_start(
                    rec_d[0:width].rearrange("(o l) -> o l", o=1),
                    rec[:, 0:width])
                rb = sm2.tile([64, AW], f32, tag="rb", name="rb")
                bcast = bass.AP(tensor=rec_d.tensor, offset=rec_d.offset,
                                ap=[[0, 64]] + list(rec_d[0:width].ap))
                nc.sync.dma_start(rb[:, 0:width], bcast)
                nc.vector.tensor_mul(attT[sec][po:po + 64, c, :],
                                     av_tile[0:E, :], rb[:, 0:width])

            # ---- out projection: one 128-row l chunk of a section ----
            def out_chunk(sec, sec0, lsc):
                ops = [psP.tile([128, 512], f32, tag="prj", name="op")
                       for _ in range(2)]
                for dhc in range(DH // 128):
                    for n2 in range(D // 512):
                        nc.tensor.matmul(
                            ops[n2][:],
                            attT[sec][:, dhc, lsc * 128:(lsc + 1) * 128],
                            wo_sb[:, dhc, n2 * 512:(n2 + 1) * 512],
                            start=(dhc == 0), stop=(dhc == DH // 128 - 1))
                row = sec0 + lsc * 128
                for n2 in range(D // 512):
                    o_sb = osp.tile([128, 512], f32, tag="o", name="o_sb")
                    nc.vector.tensor_copy(o_sb[:], ops[n2][:])
                    nc.gpsimd.dma_start(
                        out_d[row:row + 128, n2 * 512:(n2 + 1) * 512],
                        o_sb[:])

            # ================= schedule =================
            # prologue: head-pair 0's K/Q, pair 1's Q (x tiles cached).
            # wv/wo are DMA'd only now so the sync queue serves wkr/wqr/x
            # first — the first matmuls wait on those, not on wv/wo.
            kq_pass(xkT, "k", wk_r, bk_sb, kT_sb, 0, (0, 1))
            kq_pass(xqT, "q", wq_r, bq_sb, qT_sb, 0, (0, 1))
            kq_pass(xqT, "q", wq_r, bq_sb, qT_sb, 1, (0, 1))
            nc.sync.dma_start(wv_sb[:], wvT.rearrange("(c p) n -> p c n", p=128))
            nc.sync.dma_start(wo_sb[:], woT.rearrange("(c p) n -> p c n", p=128))

            # section-A interleave chunks consumed at slots j2/j3 of each
            # session (PSUM prj slots are free there; AV psum is not live).
            KQ, V = kq_pass, v_group
            during = {
                0: [lambda: V(0), lambda: V(1), lambda: V(2), lambda: V(3)],
                1: [lambda: KQ(xkT, "k", wk_r, bk_sb, kT_sb, 2, (0, 1)),
                    lambda: KQ(xqT, "q", wq_r, bq_sb, qT_sb, 2, (0, 1))],
                2: [lambda: KQ(xkT, "k", wk_r, bk_sb, kT_sb, 3, (0, 1)),
                    lambda: KQ(xqT, "q", wq_r, bq_sb, qT_sb, 3, (0, 1),
                               drop=True)],
                3: [lambda: V(4), lambda: V(5)],
                4: [lambda: V(6), lambda: V(7)],
                5: [lambda: KQ(xkT, "k", wk_r, bk_sb, kT_sb, 0, (2, 3))],
                6: [lambda: V(8), lambda: V(9)],
                7: [lambda: V(10), lambda: V(11)],
                8: [lambda: KQ(xkT, "k", wk_r, bk_sb, kT_sb, 1, (2, 3))],
                9: [lambda: KQ(xkT, "k", wk_r, bk_sb, kT_sb, 2, (2, 3))],
                10: [lambda: KQ(xkT, "k", wk_r, bk_sb, kT_sb, 3, (2, 3),
                                drop=True),
                     lambda: V(12)],
                11: [lambda: V(13), lambda: V(14)],
                12: [lambda: V(15),
                     lambda: KQ(xqT, "q", wq_r, bq_sb, qT_sb, 0, (2, 3))],
                13: [lambda: KQ(xqT, "q", wq_r, bq_sb, qT_sb, 1, (2, 3))],
                14: [lambda: KQ(xqT, "q", wq_r, bq_sb, qT_sb, 2, (2, 3))],
                15: [lambda: KQ(xqT, "q", wq_r, bq_sb, qT_sb, 3, (2, 3),
                                drop=True)],
            }
            # session 1 needs K dhc1 before its scores: emit after s0.
            after = {
                0: [lambda: KQ(xkT, "k", wk_r, bk_sb, kT_sb, 1, (0, 1))],
            }

            pending = None      # [c, stb, plist]

            def flush_half(which):
                nonlocal pending
                if pending is None:
                    return
                c, stb, plist = pending
                avs_half(c, stb, plist, which)
                if which == 1:
                    if stb == 3:
                        normalize("A", c, 0, avsb[c][0], AW)
                        normalize("A", c, 1, avsb[c][1], AW)
                    pending = None

            for si in range(16):
                stb, c = si // 4, si % 4
                dlist = during.get(si, [])
                plist = []
                for j in range(4):
                    st = 4 * stb + j
                    plist.append((st, *sc_exp_A(c, st)))
                    if j == 0:
                        flush_half(0)
                    elif j == 1:
                        flush_half(1)
                    else:
                        if dlist:
                            dlist.pop(0)()
                while dlist:
                    dlist.pop(0)()
                pending = [c, stb, plist]
                for fn_ in after.get(si, []):
                    fn_()
            flush_half(0)
            flush_half(1)

            # ---- sections B1/B2: 512-wide pairs, merged-head scores ----
            bqueue = [("A", 0, i) for i in range(AW // 128)]

            def b_pair(sec, sec0, c, predrain, last=False):
                # AV matmuls trail the exp stream by 3 steps: the previous
                # pair's normalize keeps its psV slots ~4 exps longer without
                # stalling the PE, and the boundary drains (out-projection of
                # finished sections) run after this pair's first two exps so
                # ACT is fed while attT from the last normalize settles.
                avb0 = avb1 = None
                pend = []
                nst = S // 128

                def emit_av(pst, pP):
                    nonlocal avb0, avb1
                    if avb0 is None:
                        avb0 = psV.tile([E + 1, 512], f32, tag="av",
                                        name="avb0")
                        avb1 = psV.tile([E + 1, 512], f32, tag="av",
                                        name="avb1")
                    nc.tensor.matmul(
                        avb0[:], v1_sb[:, pst, 2 * c, :], pP[:, 0:512],
                        start=(pst == 0), stop=(pst == nst - 1))
                    nc.tensor.matmul(
                        avb1[:], v1_sb[:, pst, 2 * c + 1, :],
                        pP[:, 512:1024], start=(pst == 0),
                        stop=(pst == nst - 1))

                for st in range(nst):
                    sc = psA.tile([128, 1024], f32, tag="A", name="scb")
                    nc.tensor.matmul(
                        sc[:, 0:512],
                        kT_sb[0:64, c, st * 128:(st + 1) * 128],
                        qT_sb[0:64, c, sec0:sec0 + BW],
                        start=True, stop=True)
                    nc.tensor.matmul(
                        sc[:, 512:1024],
                        kT_sb[64:128, c, st * 128:(st + 1) * 128],
                        qT_sb[64:128, c, sec0:sec0 + BW],
                        start=True, stop=True)
                    P = pp.tile([128, 1024], f16, tag="P", name="Pb")
                    nc.scalar.activation(P[:], sc[:], Exp, scale=SCALE)
                    pend.append((st, P))
                    if st == 1:
                        for _ in range(predrain):
                            if bqueue:
                                s, s0, lsc = bqueue.pop(0)
                                out_chunk(s, s0, lsc)
                    if len(pend) > (1 if last else 3):
                        emit_av(*pend.pop(0))
                while pend:
                    emit_av(*pend.pop(0))
                normalize(sec, c, 0, avb0, BW)
                normalize(sec, c, 1, avb1, BW)

            for ci_, c in enumerate(range(4)):
                b_pair("B1", 1024, c, predrain=(0 if ci_ == 0 else 2))
            bqueue.extend(("B1", 1024, i) for i in range(BW // 128))
            for ci_, c in enumerate(range(4)):
                b_pair("B2", 1536, c, predrain=3, last=(ci_ == 3))
            bqueue.extend(("B2", 1536, i) for i in range(BW // 128))
            while bqueue:
                s, s0, lsc = bqueue.pop(0)
                out_chunk(s, s0, lsc)

    nc.compile()
    if not os.environ.get("KERNEL_NO_DEDUP"):
        import concourse.mybir as mybir2
        n = _ldw_dedup(nc, mybir2)
        if os.environ.get("KERNEL_DEBUG"):
            print(f"[kernel] ldw_dedup removed {n} LDWEIGHTS")
    return nc


def _get_nc():
    if "nc" not in _compiled:
        _compiled["nc"] = _build()
    return _compiled["nc"]


def kernel(queries, keys, values, Wq, bq, Wk, bk, Wv, bv, Wo, bo):
    global last_exec_time_ns, last_results
    from concourse import bass_utils

    queries = np.asarray(queries, dtype=np.float32)
    keys = np.asarray(keys, dtype=np.float32)
    values = np.asarray(values, dtype=np.float32)
    Wq, bq = np.asarray(Wq, np.float32), np.asarray(bq, np.float32)
    Wk, bk = np.asarray(Wk, np.float32), np.asarray(bk, np.float32)
    Wv, bv = np.asarray(Wv, np.float32), np.asarray(bv, np.float32)
    Wo, bo = np.asarray(Wo, np.float32), np.asarray(bo, np.float32)

    nc = _get_nc()

    in_maps = []
    for c in range(N_CORES):
        b, g = c // HG, c % HG
        sl = slice(g * DH, (g + 1) * DH)
        in_maps.append({
            "xqT": np.ascontiguousarray(queries[b].T).astype(np.float16),
            "xkT": np.ascontiguousarray(keys[b].T).astype(np.float16),
            "xvT": np.ascontiguousarray(values[b].T).astype(np.float16),
            "wqT": np.ascontiguousarray(Wq[sl, :].T).astype(np.float16),
            "wkT": np.ascontiguousarray(Wk[sl, :].T).astype(np.float16),
            "wvT": np.ascontiguousarray(Wv[sl, :].T).astype(np.float16),
            "woT": np.ascontiguousarray(Wo[:, sl].T).astype(np.float16),
            "bq": np.ascontiguousarray(bq[sl]),
            "bk": np.ascontiguousarray(bk[sl]),
        })

    trace = bool(os.environ.get("KERNEL_TRACE"))
    if trace:
        try:
            import antenv.axon_hooks  # noqa: F401
        except ImportError:
            trace = False
    res = bass_utils.run_bass_kernel_spmd(
        nc, in_maps, core_ids=list(range(N_CORES)), trace=trace)
    last_exec_time_ns = res.exec_time_ns
    last_results = res

    const = (bo + bv @ Wo.T).astype(np.float32)
    out = np.empty((B, L, D), np.float32)
    for b in range(B):
        out[b] = (res.results[HG * b]["out"] + res.results[HG * b + 1]["out"]
                  + const)
    return out


# revision 23
# speedup vs baseline: 1.1467x; 1.1467x over previous
"""Multi-head attention layer on 8 Trainium2 NeuronCores — v2.

Sharding (zero-communication): core c -> (batch c//2, head-group c%2); each
core owns 8 of 16 heads for one batch element.  Host sums the two partial
row-parallel out-projections per batch and adds bo + bv @ Wo.T.

v2 over the ~500us baseline:
  * LDWEIGHTS dedup: post-compile pass deletes redundant InstLdweights
    (identical stationary already resident in the same PE tile region,
    wait-free, write-once whitelist).  The compiler re-emits an LDW per
    matmul (~120ns serialized each; ~180us/core total on the baseline).
  * ACT is the floor engine (~342us of exps).  Emission is software-
    pipelined so it never starves: section A (l in [0,1024)) runs c-major
    4-st sessions; each session's AV matmuls are deferred and emitted
    under the NEXT session's exps, then partial AV+sums evict to SBUF
    accumulators, freeing PSUM so K/Q/V projection chunks interleave in
    dedicated slots instead of a 139us serial prologue.
  * Sections B1/B2 (l in [1024,1536),[1536,2048)): 512-wide pairs with both
    heads' scores in one PSUM tile (one exp per st); out-projection of
    earlier sections drains at pair boundaries, shrinking the 52us tail.

dtypes: matmul operands fp16; PSUM/softmax fp32; attT/P fp16; the section-A
AV accumulators are bf16 (range for unnormalized sums, SBUF budget).

Measured: 432.2us HW exec, max rel err 2.1e-3 (gate 2e-2); same-session
baseline measured 570.8us.  Iteration history: 443.8 (sessions+dedup) ->
439.5 (B-section AV stream trails exps by 3 steps; boundary out-proj drains
move after the next pair's first two exps) -> 438.2 (late-A session loads
rebalanced to <=1 K/Q pass each) -> 432.2 (4 out-staging bufs, 8 DRAM ring
slots for the normalize round-trips).  ACT busy ~283us is the floor; the
rest is ~30us fixed NEFF-start barrier, ~16us prologue ramp, ~9us
session-0/1 K+Q deadline stall, ~15us B-boundary residue, ~35us tail.
"""

import os
import numpy as np

B, L, S = 4, 2048, 2048
D, NH, E = 1024, 16, 64
N_CORES = 8
HG = 2
LH = NH // HG          # 8 local heads
DH = LH * E            # 512
SCALE = 1.0 / np.sqrt(E)

AW = 1024              # section A width  [0, 1024)
BW = 512               # sections B1/B2   [1024,1536) [1536,2048)

_compiled = {}
last_exec_time_ns = None
last_results = None


def _ldw_dedup(nc, mybir):
    """Delete redundant InstLdweights from the compiled module.

    The PE array keeps stationary weights until the next LDWEIGHTS touching
    the same tile region.  An LDW is deletable when (a) an earlier LDW in
    the same block already loaded the identical AP into the identical
    region with no intervening overlapping load, (b) it carries no
    semaphore waits/updates, (c) the stationary is a write-once resident
    tensor (never rewritten between the two loads).
    """
    white = ("kT", "v1", "wqr", "wkr", "attT")
    deleted = 0
    for fn in nc.m.functions:
        for bb in fn.blocks:
            insts = bb.instructions
            state = []          # (r0, r1, c0, c1, key)
            todel = []
            for idx, inst in enumerate(insts):
                if isinstance(inst, mybir.InstLdweights):
                    try:
                        w0 = inst.ins[0]
                        memref = str(w0.memref)
                        tp = inst.tile_position or (0, 0)
                        ts = inst.tile_size
                        if ts is None:
                            r0, c0, r1, c1 = 0, 0, 128, 128
                        else:
                            r0, c0 = tp
                            r1, c1 = r0 + ts[0], c0 + ts[1]
                        key = (memref, w0.offset, str(w0.ap), str(w0.dtype),
                               (r0, r1, c0, c1))
                        si = inst.sync_info
                        free = si is None or (
                            len(si.on_wait) == 0 and len(si.on_update) == 0)
                        ok = any(memref.startswith(p) for p in white)
                        if free and ok and any(e[4] == key for e in state):
                            todel.append(idx)
                            continue
                        state = [e for e in state
                                 if not (e[0] < r1 and r0 < e[1]
                                         and e[2] < c1 and c0 < e[3])]
                        state.append((r0, r1, c0, c1, key))
                    except Exception:
                        state = []
                elif isinstance(inst, mybir.InstMatmult):
                    if inst.is_transpose:
                        state = []
            for idx in reversed(todel):
                del insts[idx]
            deleted += len(todel)
    return deleted


def _build():
    import concourse.bass as bass
    import concourse.mybir as mybir
    import concourse.tile as tile
    from concourse import bacc

    f32 = mybir.dt.float32
    f16 = mybir.dt.float16
    bf16 = mybir.dt.bfloat16

    nc = bacc.Bacc("TRN2", target_bir_lowering=False, debug=False,
                   num_devices=N_CORES)

    xqT = nc.dram_tensor("xqT", [D, L], f16, kind="ExternalInput").ap()
    xkT = nc.dram_tensor("xkT", [D, S], f16, kind="ExternalInput").ap()
    xvT = nc.dram_tensor("xvT", [D, S], f16, kind="ExternalInput").ap()
    wqT = nc.dram_tensor("wqT", [D, DH], f16, kind="ExternalInput").ap()
    wkT = nc.dram_tensor("wkT", [D, DH], f16, kind="ExternalInput").ap()
    wvT = nc.dram_tensor("wvT", [D, DH], f16, kind="ExternalInput").ap()
    woT = nc.dram_tensor("woT", [DH, D], f16, kind="ExternalInput").ap()
    bq_d = nc.dram_tensor("bq", [DH], f32, kind="ExternalInput").ap()
    bk_d = nc.dram_tensor("bk", [DH], f32, kind="ExternalInput").ap()
    out_d = nc.dram_tensor("out", [L, D], f32, kind="ExternalOutput").ap()

    Exp = mybir.ActivationFunctionType.Exp
    ADD = mybir.AluOpType.add

    with tile.TileContext(nc) as tc:
        with (
            tc.tile_pool(name="res", bufs=1) as res,
            tc.tile_pool(name="xsk", bufs=32) as xsk,
            tc.tile_pool(name="xsv", bufs=8) as xsv,
            tc.tile_pool(name="pp", bufs=12) as pp,
            tc.tile_pool(name="os", bufs=4) as osp,
            tc.tile_pool(name="sm", bufs=1) as sm,
            tc.tile_pool(name="sm2", bufs=2) as sm2,
            tc.tile_pool(name="dr", bufs=8, space="DRAM") as dr,
            tc.tile_pool(name="psA", bufs=2, space="PSUM") as psA,
            tc.tile_pool(name="psV", bufs=2, space="PSUM") as psV,
            tc.tile_pool(name="psP", bufs=2, space="PSUM") as psP,
        ):
            # ---- resident weights / constants ----
            bq_sb = res.tile([128, DH // 128], f32, tag="bq")
            bk_sb = res.tile([128, DH // 128], f32, tag="bk")
            nc.sync.dma_start(bq_sb[:], bq_d.rearrange("(c p) -> p c", p=128))
            nc.sync.dma_start(bk_sb[:], bk_d.rearrange("(c p) -> p c", p=128))
            wo_sb = res.tile([128, DH // 128, D], f16, tag="wo")
            nc.sync.dma_start(wo_sb[:], woT.rearrange("(c p) n -> p c n", p=128))
            wv_sb = res.tile([128, D // 128, DH], f16, tag="wv")
            nc.sync.dma_start(wv_sb[:], wvT.rearrange("(c p) n -> p c n", p=128))
            wq_r = res.tile([128, D // 128, DH], f16, tag="wqr", name="wqr")
            nc.sync.dma_start(wq_r[:], wqT.rearrange("(c p) n -> p c n", p=128))
            wk_r = res.tile([128, D // 128, DH], f16, tag="wkr", name="wkr")
            nc.sync.dma_start(wk_r[:], wkT.rearrange("(c p) n -> p c n", p=128))
            ones_f = res.tile([128, 128], f32, tag="onesf")
            nc.vector.memset(ones_f[:], 1.0)

            qT_sb = res.tile([128, DH // 128, L], f16, tag="qT", name="qT")
            kT_sb = res.tile([128, DH // 128, S], f16, tag="kT", name="kT")
            v1_sb = res.tile([128, S // 128, LH, E + 1], f16, tag="v1",
                             name="v1")
            nc.vector.tensor_copy(
                v1_sb[:, :, :, E:E + 1],
                ones_f[:, 0:S // 128 * LH].rearrange(
                    "p (s h o) -> p s h o", h=LH, o=1))

            attT = {
                "A": res.tile([128, DH // 128, AW], f16, tag="attTA",
                              name="attTA"),
                "B1": res.tile([128, DH // 128, BW], f16, tag="attTB1",
                               name="attTB1"),
                "B2": res.tile([128, DH // 128, BW], f16, tag="attTB2",
                               name="attTB2"),
            }
            # section-A AV accumulators (values rows 0:E, sums row E)
            avsb = [[res.tile([E + 1, AW], bf16, tag=f"avsb{c}{h}",
                              name=f"avsb{c}{h}")
                     for h in range(2)] for c in range(4)]

            # ---- x staging ----
            xcache = {}

            def load_x(src, nm, d, bl):
                k = (nm, d, bl)
                if k not in xcache:
                    pool = xsv if nm == "v" else xsk
                    t = pool.tile([128, 512], f16, tag="xs", name="xt")
                    nc.sync.dma_start(
                        t[:], src[d * 128:(d + 1) * 128,
                                  bl * 512:(bl + 1) * 512])
                    xcache[k] = t
                return xcache[k]

            def drop_x(nm, bls):
                for d in range(8):
                    for bl in bls:
                        xcache.pop((nm, d, bl), None)

            # ---- fused K/Q projection: one dhc chunk over a bl pair ----
            def kq_pass(src, nm, w_r, b_sb, dst, dhc, bls, drop=False):
                prj = [psP.tile([128, 512], f32, tag="prj", name="prj")
                       for _ in bls]
                for d in range(8):
                    for i, bl in enumerate(bls):
                        xt = load_x(src, nm, d, bl)
                        nc.tensor.matmul(
                            prj[i][:], w_r[:, d, dhc * 128:(dhc + 1) * 128],
                            xt[:], start=(d == 0), stop=(d == 7))
                for i, bl in enumerate(bls):
                    nc.vector.tensor_scalar_add(
                        out=dst[:, dhc, bl * 512:(bl + 1) * 512],
                        in0=prj[i][:], scalar1=b_sb[:, dhc:dhc + 1])
                if drop:
                    drop_x(nm, bls)

            # ---- V projection: one st tile (128 tokens, all local heads) --
            def v_group(st):
                bl = st // 4
                vp = psP.tile([128, DH], f32, tag="prj", name="vp")
                for d in range(8):
                    xt = load_x(xvT, "v", d, bl)
                    nc.tensor.matmul(
                        vp[:], xt[:, (st % 4) * 128:(st % 4 + 1) * 128],
                        wv_sb[:, d, :], start=(d == 0), stop=(d == 7))
                nc.vector.tensor_copy(
                    out=v1_sb[:, st, :, 0:E],
                    in_=vp.rearrange("p (h e) -> p h e", h=LH))
                if st % 4 == 3:
                    drop_x("v", (bl,))

            # ---- section A scores + exp (two separate head tiles) ----
            def sc_exp_A(c, st):
                sc0 = psA.tile([128, AW], f32, tag="A", name="sc0")
                sc1 = psA.tile([128, AW], f32, tag="A", name="sc1")
                for nh in range(AW // 512):
                    lo = nh * 512
                    nc.tensor.matmul(
                        sc0[:, lo:lo + 512],
                        kT_sb[0:64, c, st * 128:(st + 1) * 128],
                        qT_sb[0:64, c, lo:lo + 512], start=True, stop=True)
                    nc.tensor.matmul(
                        sc1[:, lo:lo + 512],
                        kT_sb[64:128, c, st * 128:(st + 1) * 128],
                        qT_sb[64:128, c, lo:lo + 512], start=True, stop=True)
                P0 = pp.tile([128, AW], f16, tag="P", name="P0")
                nc.scalar.activation(P0[:], sc0[:], Exp, scale=SCALE)
                P1 = pp.tile([128, AW], f16, tag="P", name="P1")
                nc.scalar.activation(P1[:], sc1[:], Exp, scale=SCALE)
                return P0, P1

            def avs_half(c, stb, plist, half):
                # AV accumulation for l-columns [half*512,(half+1)*512) over
                # the session's 4 st tiles, then evict-accumulate to SBUF.
                o = half * 512
                av0 = psV.tile([E + 1, 512], f32, tag="av", name="av0")
                av1 = psV.tile([E + 1, 512], f32, tag="av", name="av1")
                n = len(plist)
                for i in range(n):
                    st, P0, P1 = plist[i]
                    fs, ls = (i == 0), (i == n - 1)
                    nc.tensor.matmul(
                        av0[:], v1_sb[:, st, 2 * c, :],
                        P0[:, o:o + 512], start=fs, stop=ls)
                    nc.tensor.matmul(
                        av1[:], v1_sb[:, st, 2 * c + 1, :],
                        P1[:, o:o + 512], start=fs, stop=ls)
                for h, av in ((0, av0), (1, av1)):
                    if stb == 0:
                        nc.vector.tensor_copy(
                            avsb[c][h][:, o:o + 512], av[:])
                    else:
                        nc.vector.tensor_tensor(
                            out=avsb[c][h][:, o:o + 512],
                            in0=avsb[c][h][:, o:o + 512], in1=av[:],
                            op=ADD)

            # ---- normalize: attT[po:po+64, c, :] = av[0:E] / av[E] ----
            def normalize(sec, c, h, av_tile, width):
                po = h * 64
                sums0 = sm.tile([1, AW], f32, tag="sums", name="sums0")
                nc.vector.tensor_copy(sums0[:, 0:width], av_tile[E:E + 1, :])
                rec = sm.tile([1, AW], f32, tag="rec", name="rec")
                scr = sm.tile([1, AW], f32, tag="scr", name="scr")
                nc.vector.reciprocal_approx_accurate(
                    rec[:, 0:width], sums0[:, 0:width], scr[:, 0:width])
                rec_d = dr.tile([AW], f32, tag="recd", name="rec_d")
                nc.sync.dma_start(
                    rec_d[0:width].rearrange("(o l) -> o l", o=1),
                    rec[:, 0:width])
                rb = sm2.tile([64, AW], f32, tag="rb", name="rb")
                bcast = bass.AP(tensor=rec_d.tensor, offset=rec_d.offset,
                                ap=[[0, 64]] + list(rec_d[0:width].ap))
                nc.sync.dma_start(rb[:, 0:width], bcast)
                nc.vector.tensor_mul(attT[sec][po:po + 64, c, :],
                                     av_tile[0:E, :], rb[:, 0:width])

            # ---- out projection: one 128-row l chunk of a section ----
            def out_chunk(sec, sec0, lsc):
                ops = [psP.tile([128, 512], f32, tag="prj", name="op")
                       for _ in range(2)]
                for dhc in range(DH // 128):
                    for n2 in range(D // 512):
                        nc.tensor.matmul(
                            ops[n2][:],
                            attT[sec][:, dhc, lsc * 128:(lsc + 1) * 128],
                            wo_sb[:, dhc, n2 * 512:(n2 + 1) * 512],
                            start=(dhc == 0), stop=(dhc == DH // 128 - 1))
                row = sec0 + lsc * 128
                for n2 in range(D // 512):
                    o_sb = osp.tile([128, 512], f32, tag="o", name="o_sb")
                    nc.vector.tensor_copy(o_sb[:], ops[n2][:])
                    nc.gpsimd.dma_start(
                        out_d[row:row + 128, n2 * 512:(n2 + 1) * 512],
                        o_sb[:])

            # ================= schedule =================
            # prologue: head-pair 0's K and Q for section A
            kq_pass(xkT, "k", wk_r, bk_sb, kT_sb, 0, (0, 1))
            kq_pass(xqT, "q", wq_r, bq_sb, qT_sb, 0, (0, 1))

            # section-A interleave chunks consumed at slots j2/j3 of each
            # session (PSUM prj slots are free there; AV psum is not live).
            KQ, V = kq_pass, v_group
            during = {
                0: [lambda: V(0), lambda: V(1), lambda: V(2), lambda: V(3)],
                1: [lambda: KQ(xkT, "k", wk_r, bk_sb, kT_sb, 2, (0, 1)),
                    lambda: KQ(xqT, "q", wq_r, bq_sb, qT_sb, 2, (0, 1))],
                2: [lambda: KQ(xkT, "k", wk_r, bk_sb, kT_sb, 3, (0, 1)),
                    lambda: KQ(xqT, "q", wq_r, bq_sb, qT_sb, 3, (0, 1),
                               drop=True)],
                3: [lambda: V(4), lambda: V(5)],
                4: [lambda: V(6), lambda: V(7)],
                5: [lambda: KQ(xkT, "k", wk_r, bk_sb, kT_sb, 0, (2, 3))],
                6: [lambda: V(8), lambda: V(9)],
                7: [lambda: V(10), lambda: V(11)],
                8: [lambda: KQ(xkT, "k", wk_r, bk_sb, kT_sb, 1, (2, 3))],
                9: [lambda: KQ(xkT, "k", wk_r, bk_sb, kT_sb, 2, (2, 3))],
                10: [lambda: KQ(xkT, "k", wk_r, bk_sb, kT_sb, 3, (2, 3),
                                drop=True),
                     lambda: V(12)],
                11: [lambda: V(13), lambda: V(14)],
                12: [lambda: V(15),
                     lambda: KQ(xqT, "q", wq_r, bq_sb, qT_sb, 0, (2, 3))],
                13: [lambda: KQ(xqT, "q", wq_r, bq_sb, qT_sb, 1, (2, 3))],
                14: [lambda: KQ(xqT, "q", wq_r, bq_sb, qT_sb, 2, (2, 3))],
                15: [lambda: KQ(xqT, "q", wq_r, bq_sb, qT_sb, 3, (2, 3),
                                drop=True)],
            }
            # session 1 needs K/Q dhc1 before its scores: emit after s0.
            after = {
                0: [lambda: KQ(xkT, "k", wk_r, bk_sb, kT_sb, 1, (0, 1)),
                    lambda: KQ(xqT, "q", wq_r, bq_sb, qT_sb, 1, (0, 1))],
            }

            pending = None      # [c, stb, plist]

            def flush_half(which):
                nonlocal pending
                if pending is None:
                    return
                c, stb, plist = pending
                avs_half(c, stb, plist, which)
                if which == 1:
                    if stb == 3:
                        normalize("A", c, 0, avsb[c][0], AW)
                        normalize("A", c, 1, avsb[c][1], AW)
                    pending = None

            for si in range(16):
                stb, c = si // 4, si % 4
                dlist = during.get(si, [])
                plist = []
                for j in range(4):
                    st = 4 * stb + j
                    plist.append((st, *sc_exp_A(c, st)))
                    if j == 0:
                        flush_half(0)
                    elif j == 1:
                        flush_half(1)
                    else:
                        if dlist:
                            dlist.pop(0)()
                while dlist:
                    dlist.pop(0)()
                pending = [c, stb, plist]
                for fn_ in after.get(si, []):
                    fn_()
            flush_half(0)
            flush_half(1)

            # ---- sections B1/B2: 512-wide pairs, merged-head scores ----
            bqueue = [("A", 0, i) for i in range(AW // 128)]

            def b_pair(sec, sec0, c, predrain, last=False):
                # AV matmuls trail the exp stream by 3 steps: the previous
                # pair's normalize keeps its psV slots ~4 exps longer without
                # stalling the PE, and the boundary drains (out-projection of
                # finished sections) run after this pair's first two exps so
                # ACT is fed while attT from the last normalize settles.
                avb0 = avb1 = None
                pend = []
                nst = S // 128

                def emit_av(pst, pP):
                    nonlocal avb0, avb1
                    if avb0 is None:
                        avb0 = psV.tile([E + 1, 512], f32, tag="av",
                                        name="avb0")
                        avb1 = psV.tile([E + 1, 512], f32, tag="av",
                                        name="avb1")
                    nc.tensor.matmul(
                        avb0[:], v1_sb[:, pst, 2 * c, :], pP[:, 0:512],
                        start=(pst == 0), stop=(pst == nst - 1))
                    nc.tensor.matmul(
                        avb1[:], v1_sb[:, pst, 2 * c + 1, :],
                        pP[:, 512:1024], start=(pst == 0),
                        stop=(pst == nst - 1))

                for st in range(nst):
                    sc = psA.tile([128, 1024], f32, tag="A", name="scb")
                    nc.tensor.matmul(
                        sc[:, 0:512],
                        kT_sb[0:64, c, st * 128:(st + 1) * 128],
                        qT_sb[0:64, c, sec0:sec0 + BW],
                        start=True, stop=True)
                    nc.tensor.matmul(
                        sc[:, 512:1024],
                        kT_sb[64:128, c, st * 128:(st + 1) * 128],
                        qT_sb[64:128, c, sec0:sec0 + BW],
                        start=True, stop=True)
                    P = pp.tile([128, 1024], f16, tag="P", name="Pb")
                    nc.scalar.activation(P[:], sc[:], Exp, scale=SCALE)
                    pend.append((st, P))
                    if st == 1:
                        for _ in range(predrain):
                            if bqueue:
                                s, s0, lsc = bqueue.pop(0)
                                out_chunk(s, s0, lsc)
                    if len(pend) > (1 if last else 3):
                        emit_av(*pend.pop(0))
                while pend:
                    emit_av(*pend.pop(0))
                normalize(sec, c, 0, avb0, BW)
                normalize(sec, c, 1, avb1, BW)

            for ci_, c in enumerate(range(4)):
                b_pair("B1", 1024, c, predrain=(0 if ci_ == 0 else 2))
            bqueue.extend(("B1", 1024, i) for i in range(BW // 128))
            for ci_, c in enumerate(range(4)):
                b_pair("B2", 1536, c, predrain=3, last=(ci_ == 3))
            bqueue.extend(("B2", 1536, i) for i in range(BW // 128))
            while bqueue:
                s, s0, lsc = bqueue.pop(0)
                out_chunk(s, s0, lsc)

    nc.compile()
    if not os.environ.get("KERNEL_NO_DEDUP"):
        import concourse.mybir as mybir2
        n = _ldw_dedup(nc, mybir2)
        if os.environ.get("KERNEL_DEBUG"):
            print(f"[kernel] ldw_dedup removed {n} LDWEIGHTS")
    return nc


def _get_nc():
    if "nc" not in _compiled:
        _compiled["nc"] = _build()
    return _compiled["nc"]


def kernel(queries, keys, values, Wq, bq, Wk, bk, Wv, bv, Wo, bo):
    global last_exec_time_ns, last_results
    from concourse import bass_utils

    queries = np.asarray(queries, dtype=np.float32)
    keys = np.asarray(keys, dtype=np.float32)
    values = np.asarray(values, dtype=np.float32)
    Wq, bq = np.asarray(Wq, np.float32), np.asarray(bq, np.float32)
    Wk, bk = np.asarray(Wk, np.float32), np.asarray(bk, np.float32)
    Wv, bv = np.asarray(Wv, np.float32), np.asarray(bv, np.float32)
    Wo, bo = np.asarray(Wo, np.float32), np.asarray(bo, np.float32)

    nc = _get_nc()

    in_maps = []
    for c in range(N_CORES):
        b, g = c // HG, c % HG
        sl = slice(g * DH, (g + 1) * DH)
        in_maps.append({
            "xqT": np.ascontiguousarray(queries[b].T).astype(np.float16),
            "xkT": np.ascontiguousarray(keys[b].T).astype(np.float16),
            "xvT": np.ascontiguousarray(values[b].T).astype(np.float16),
            "wqT": np.ascontiguousarray(Wq[sl, :].T).astype(np.float16),
            "wkT": np.ascontiguousarray(Wk[sl, :].T).astype(np.float16),
            "wvT": np.ascontiguousarray(Wv[sl, :].T).astype(np.float16),
            "woT": np.ascontiguousarray(Wo[:, sl].T).astype(np.float16),
            "bq": np.ascontiguousarray(bq[sl]),
            "bk": np.ascontiguousarray(bk[sl]),
        })

    trace = bool(os.environ.get("KERNEL_TRACE"))
    if trace:
        try:
            import antenv.axon_hooks  # noqa: F401
        except ImportError:
            trace = False
    res = bass_utils.run_bass_kernel_spmd(
        nc, in_maps, core_ids=list(range(N_CORES)), trace=trace)
    last_exec_time_ns = res.exec_time_ns
    last_results = res

    const = (bo + bv @ Wo.T).astype(np.float32)
    out = np.empty((B, L, D), np.float32)
    for b in range(B):
        out[b] = (res.results[HG * b]["out"] + res.results[HG * b + 1]["out"]
                  + const)
    return out
